# revision 6
# baseline (speedup 1.0000x reference)
"""Trainium2 Bass kernel for an encoder-decoder (S2S) transformer.

Distribution: 8 NeuronCores = 4 data-parallel groups (batch B=4) x 2-way
Megatron tensor-parallel within each same-SEngine core pair.  Per TP pair,
attention heads (qkv/out) and ffn (ff1/ff2) are sharded; partials combine
with a 2-core AllReduce after the attention out-projection and after ff2.

Matmuls run in bf16 on the TensorEngine (fp32 PSUM accumulation); the
residual stream and layernorm statistics stay fp32.  Activations are
SBUF-resident for the whole pass; only weights stream from HBM.

Layout conventions (per core, P=128):
  x_nat[t]  [P, D]   fp32   natural tokens-on-partitions residual stream
  x_T       [P, ND*T] bf16  transposed: chunk d cols [d*T:(d+1)*T]
  q_T/k_T   [P, NQK*T] bf16 rows = sharded head dims, chunk m = heads 2m,2m+1
  v_nat     [P, NT*DL] bf16 t-chunk cols [t*DL:(t+1)*DL]
  attn_T    [P, (DL/P)*T] bf16  context, transposed (rows = sharded dims)
  h_T       [P, NFF*T] bf16 ffn hidden, transposed
"""

import os
import sys

for _p in ("/opt/trn_rl_repo", "/root/.axon_site/_ro/trn_rl_repo"):
    if os.path.isdir(_p) and _p not in sys.path:
        sys.path.insert(0, _p)

import numpy as np
import ml_dtypes

import concourse.bass as bass
import concourse.bacc as bacc
import concourse.tile as tile
from concourse import mybir
from concourse.bass import IndirectOffsetOnAxis
from concourse.masks import make_identity, make_causal_mask

BF16 = ml_dtypes.bfloat16
F32 = mybir.dt.float32
BF = mybir.dt.bfloat16
I32 = mybir.dt.int32
AF = mybir.ActivationFunctionType
ALU = mybir.AluOpType
AX = mybir.AxisListType

P = 128


class Cfg:
    def __init__(self, B, Q, T, D, H, V, L, FF, TP, n_cores, flags=frozenset()):
        self.B, self.Q, self.T, self.D, self.H, self.V, self.L, self.FF = \
            B, Q, T, D, H, V, L, FF
        self.TP, self.n_cores = TP, n_cores
        self.E = D // Q
        self.HD = D // H
        assert self.HD == 64, "head packing assumes head_dim 64"
        assert self.E == P, "per-quantizer embedding dim must be 128"
        self.SCALE = 1.0 / float(np.sqrt(self.HD))
        self.DL = D // TP
        self.FFL = FF // TP
        self.HL = H // TP
        self.NT = T // P
        self.ND = D // P
        self.NQK = self.DL // P
        self.NO = self.DL // P      # attn_T chunks
        self.NFF = self.FFL // P
        assert self.HL % 2 == 0, "needs an even number of local heads"
        self.flags = frozenset(flags)

    def key(self):
        return (self.B, self.Q, self.T, self.D, self.H, self.V, self.L,
                self.FF, self.TP, self.n_cores, tuple(sorted(self.flags)))


# --------------------------------------------------------------------------
# program builder
# --------------------------------------------------------------------------

def build_program(c: Cfg):
    nc = bacc.Bacc(None, target_bir_lowering=False, num_devices=c.n_cores)

    def din(name, shape, dt=BF):
        return nc.dram_tensor(name, shape, dt, kind="ExternalInput")

    codes_in = din("codes_in", [c.Q, c.T], I32)
    codes_tgt = din("codes_tgt", [c.Q, c.T], I32)
    tok_emb = [din(f"tok_emb_{q}", [c.V, c.E], F32) for q in range(c.Q)]
    pos = din("pos", [c.T, c.D], F32)

    w = {}
    for l in range(c.L):
        for nm, sh in [
                (f"e_qkv_{l}", [c.D, 3 * c.DL]), (f"e_out_{l}", [c.DL, c.D]),
                (f"e_ff1_{l}", [c.D, c.FFL]), (f"e_ff2_{l}", [c.FFL, c.D]),
                (f"d_sqkv_{l}", [c.D, 3 * c.DL]), (f"d_sout_{l}", [c.DL, c.D]),
                (f"d_cqkv_{l}", [c.D, 3 * c.DL]), (f"d_cout_{l}", [c.DL, c.D]),
                (f"d_ff1_{l}", [c.D, c.FFL]), (f"d_ff2_{l}", [c.FFL, c.D])]:
            w[nm] = din(nm, sh)
    w["head_t"] = din("head_t", [c.E, c.Q * c.V])

    opt = {}
    for nm in c.flags:
        if "_qkv_b_" in nm or "_sqkv_b_" in nm or "_cqkv_b_" in nm:
            opt[nm] = din(nm, [3 * c.DL], F32)
        elif "_ff1_b_" in nm:
            opt[nm] = din(nm, [c.FFL], F32)
        elif nm == "head_b":
            opt[nm] = din(nm, [P, c.Q * c.V], F32)
        else:
            opt[nm] = din(nm, [P, c.D], F32)

    logits = nc.dram_tensor("logits", [c.Q, c.T, c.V], F32,
                            kind="ExternalOutput")

    groups = ([[g * c.TP + i for i in range(c.TP)]
               for g in range(c.n_cores // c.TP)] if c.TP > 1 else None)

    with tile.TileContext(nc) as tc:
        _emit(nc, tc, c, codes_in, codes_tgt, tok_emb, pos, w, opt, logits,
              groups)
    nc.compile()
    return nc


def _emit(nc, tc, c, codes_in, codes_tgt, tok_emb, pos, w, opt, logits,
          groups):
    from contextlib import ExitStack
    es = ExitStack()
    pool = lambda name, bufs, space="SBUF": es.enter_context(
        tc.tile_pool(name=name, bufs=bufs, space=space))

    const = pool("const", 1)
    persist = pool("persist", 1)
    wpool = pool("wpool", 3)
    act = pool("act", 3)
    scratch = pool("scratch", 2)
    dram = pool("dram", 4, space="DRAM")
    ps_proj = pool("ps_proj", 2, space="PSUM")
    ps_s = pool("ps_s", 2, space="PSUM")
    ps_av = pool("ps_av", 2, space="PSUM")
    ps_tr = pool("ps_tr", 2, space="PSUM")

    # constants
    ident_f = const.tile([P, P], F32, name="ident_f")
    make_identity(nc, ident_f[:])
    ident_b = const.tile([P, P], BF, name="ident_b")
    make_identity(nc, ident_b[:])
    caus = const.tile([P, P], F32, name="caus")
    make_causal_mask(nc, caus[:], mask_val=-1e9)
    eps_t = const.tile([P, 1], F32, name="eps_t")
    nc.vector.memset(eps_t[:], 1e-5)

    opt_sb = {}
    for nm in opt:
        if "qkv_b_" in nm:
            t = const.tile([P, 3 * c.NQK], F32, name=f"sb_{nm}")
            nc.sync.dma_start(out=t[:],
                              in_=opt[nm].rearrange("(m p) -> p m", p=P))
        elif "_ff1_b_" in nm:
            t = const.tile([P, c.NFF], F32, name=f"sb_{nm}")
            nc.sync.dma_start(out=t[:],
                              in_=opt[nm].rearrange("(m p) -> p m", p=P))
        else:
            sh = [P, c.Q * c.V] if nm == "head_b" else [P, c.D]
            t = const.tile(sh, F32, name=f"sb_{nm}")
            nc.sync.dma_start(out=t[:], in_=opt[nm][:])
        opt_sb[nm] = t

    # persistent activations
    x_nat = [persist.tile([P, c.D], F32, name=f"x_{t}", tag=f"x_{t}")
             for t in range(c.NT)]
    x_T = persist.tile([P, c.ND * c.T], BF, name="x_T", tag="x_T")
    mem_T = persist.tile([P, c.ND * c.T], BF, name="mem_T", tag="mem_T")
    q_T = persist.tile([P, c.NQK * c.T], BF, name="q_T", tag="q_T")
    k_T = persist.tile([P, c.NQK * c.T], BF, name="k_T", tag="k_T")
    v_nat = persist.tile([P, c.NT * c.DL], BF, name="v_nat", tag="v_nat")
    attn_T = persist.tile([P, c.NO * c.T], BF, name="attn_T", tag="attn_T")
    h_T = persist.tile([P, c.NFF * c.T], BF, name="h_T", tag="h_T")

    # ---------------- helpers ----------------
    def transpose_x_into(dst, copy_eng):
        """dst[:, d*T + t*P : +P] = x_nat[t][:, d*P:+P].T (fp32 -> bf16)."""
        for t in range(c.NT):
            for d in range(c.ND):
                pt = ps_tr.tile([P, P], F32, tag="ps_tr", name="pt")
                nc.tensor.transpose(pt[:], x_nat[t][:, d * P:(d + 1) * P],
                                    ident_f[:])
                if copy_eng is nc.scalar:
                    nc.scalar.copy(
                        out=dst[:, d * c.T + t * P: d * c.T + t * P + P],
                        in_=pt[:])
                else:
                    copy_eng.tensor_copy(
                        out=dst[:, d * c.T + t * P: d * c.T + t * P + P],
                        in_=pt[:])

    def embed(codes):
        for t in range(c.NT):
            ptile = scratch.tile([P, c.D], F32, tag="pos", name="ptile")
            nc.sync.dma_start(out=ptile[:], in_=pos[t * P:(t + 1) * P, :])
            for q in range(c.Q):
                idx = scratch.tile([P, 1], I32, tag="idx", name="idx")
                nc.sync.dma_start(out=idx[:], in_=codes[q, t * P:(t + 1) * P])
                nc.gpsimd.indirect_dma_start(
                    out=x_nat[t][:, q * c.E:(q + 1) * c.E],
                    out_offset=None,
                    in_=tok_emb[q][:],
                    in_offset=IndirectOffsetOnAxis(ap=idx[:, :1], axis=0))
            nc.vector.tensor_tensor(out=x_nat[t][:], in0=x_nat[t][:],
                                    in1=ptile[:], op=ALU.add)

    def layernorm(gname, bname):
        for t in range(c.NT):
            xt = x_nat[t]
            s1 = scratch.tile([P, 1], F32, tag="lnstat", name="s1", bufs=8)
            s2 = scratch.tile([P, 1], F32, tag="lnstat", name="s2", bufs=8)
            sq = scratch.tile([P, c.D], BF, tag="lnsq", name="sq")
            nc.vector.reduce_sum(out=s1[:], in_=xt[:], axis=AX.X)
            nc.scalar.activation(sq[:], xt[:], AF.Square, accum_out=s2[:])
            mean = scratch.tile([P, 1], F32, tag="lnstat", name="mean", bufs=8)
            var = scratch.tile([P, 1], F32, tag="lnstat", name="var", bufs=8)
            m2 = scratch.tile([P, 1], F32, tag="lnstat", name="m2", bufs=8)
            nc.vector.tensor_scalar_mul(mean[:], s1[:], 1.0 / c.D)
            nc.vector.tensor_scalar_mul(var[:], s2[:], 1.0 / c.D)
            nc.vector.tensor_tensor(out=m2[:], in0=mean[:], in1=mean[:],
                                    op=ALU.mult)
            nc.vector.tensor_tensor(out=var[:], in0=var[:], in1=m2[:],
                                    op=ALU.subtract)
            rstd = scratch.tile([P, 1], F32, tag="lnstat", name="rstd", bufs=8)
            nc.scalar.activation(rstd[:], var[:], AF.Sqrt, bias=eps_t[:])
            nc.vector.reciprocal(rstd[:], rstd[:])
            nc.vector.tensor_scalar(out=xt[:], in0=xt[:], scalar1=mean[:],
                                    scalar2=rstd[:], op0=ALU.subtract,
                                    op1=ALU.mult)
            if gname in opt_sb:
                nc.vector.tensor_tensor(out=xt[:], in0=xt[:],
                                        in1=opt_sb[gname][:], op=ALU.mult)
            if bname in opt_sb:
                nc.vector.tensor_tensor(out=xt[:], in0=xt[:],
                                        in1=opt_sb[bname][:], op=ALU.add)

    def allreduce_residual(parts):
        """parts: per-t bf16 [P, D] tiles holding this core's partial sums."""
        if groups is None:
            for t in range(c.NT):
                nc.vector.tensor_tensor(out=x_nat[t][:], in0=x_nat[t][:],
                                        in1=parts[t][:], op=ALU.add)
            return
        arin = dram.tile([c.T, c.D], BF, tag="arin", name="arin")
        arout = dram.tile([c.T, c.D], BF, tag="arout", name="arout")
        for t in range(c.NT):
            nc.sync.dma_start(out=arin[t * P:(t + 1) * P, :], in_=parts[t][:])
        nc.gpsimd.collective_compute(
            "AllReduce", ALU.add, replica_groups=groups,
            ins=[arin[:].opt()], outs=[arout[:].opt()])
        for t in range(c.NT):
            red = scratch.tile([P, c.D], BF, tag="ar_red", name="red")
            nc.sync.dma_start(out=red[:], in_=arout[t * P:(t + 1) * P, :])
            nc.vector.tensor_tensor(out=x_nat[t][:], in0=x_nat[t][:],
                                    in1=red[:], op=ALU.add)

    def proj_rows(wname, bname, src_T, row_base, dst):
        """dst[:, m*T:(m+1)*T] rows [row_base + m*P ...] of W.T @ src."""
        for m in range(c.NQK):
            mg = row_base // P + m
            wt = wpool.tile([P, c.ND * P], BF, tag="wqkv", name="wt")
            nc.sync.dma_start(
                out=wt[:].rearrange("p (nd m) -> p nd m", m=P),
                in_=w[wname][:, row_base + m * P: row_base + (m + 1) * P]
                .rearrange("(nd p) m -> p nd m", p=P))
            ps = ps_proj.tile([P, c.T], F32, tag="ps_proj", name="ps")
            for k in range(c.ND):
                nc.tensor.matmul(ps[:], wt[:, k * P:(k + 1) * P],
                                 src_T[:, k * c.T:(k + 1) * c.T],
                                 start=(k == 0), stop=(k == c.ND - 1))
            col = m * c.T
            if bname in opt_sb:
                nc.vector.tensor_scalar(
                    out=dst[:, col:col + c.T], in0=ps[:],
                    scalar1=opt_sb[bname][:, mg:mg + 1], scalar2=None,
                    op0=ALU.add)
            else:
                nc.scalar.copy(out=dst[:, col:col + c.T], in_=ps[:])

    def proj_v(wname, bname, src_T):
        wts = []
        for k in range(c.ND):
            wt = wpool.tile([P, c.DL], BF, tag=f"wv_{k}", name="wt", bufs=1)
            nc.sync.dma_start(
                out=wt[:],
                in_=w[wname][k * P:(k + 1) * P, 2 * c.DL:3 * c.DL])
            wts.append(wt)
        for t in range(c.NT):
            ps = ps_proj.tile([P, c.DL], F32, tag="ps_proj", name="ps")
            for k in range(c.ND):
                nc.tensor.matmul(
                    ps[:], src_T[:, k * c.T + t * P: k * c.T + t * P + P],
                    wts[k][:], start=(k == 0), stop=(k == c.ND - 1))
            dst = v_nat[:, t * c.DL:(t + 1) * c.DL]
            nc.scalar.copy(out=dst, in_=ps[:])
            if bname in opt_sb:
                bb = scratch.tile([P, c.DL], BF, tag="vb", name="bb")
                # broadcast add: bias lives as per-partition [P, 3*NQK];
                # v rows are free-dim here -> need a broadcast tile instead.
                # Host provides *_vbias broadcast input in that case.
                raise NotImplementedError("nonzero v bias unsupported")

    def attention(causal):
        for hp in range(c.HL // 2):
            ps_pair = ps_av.tile([P, c.T], F32, tag="ps_av", name="ps_pair")
            for sub in range(2):
                h = hp * 2 + sub
                m = h // 2
                po = 64 * (h % 2)
                at_tiles = [act.tile([P, c.T], BF, tag=f"AT{tk}", name="at",
                                     bufs=2) for tk in range(c.NT)]
                for tq in range(c.NT):
                    ntk = (tq + 1) * P if causal else c.T
                    pss = ps_s.tile([P, c.T], F32, tag="ps_s", name="pss")
                    nc.tensor.matmul(
                        pss[:, :ntk],
                        q_T[po:po + 64, m * c.T + tq * P: m * c.T + tq * P + P],
                        k_T[po:po + 64, m * c.T: m * c.T + ntk],
                        start=True, stop=True)
                    if causal:
                        nc.vector.tensor_tensor(
                            out=pss[:, tq * P:(tq + 1) * P],
                            in0=pss[:, tq * P:(tq + 1) * P],
                            in1=caus[:], op=ALU.add)
                    A = act.tile([P, c.T], BF, tag="A", name="A")
                    den = scratch.tile([P, 1], F32, tag="den", name="den",
                                       bufs=4)
                    nc.scalar.activation(A[:, :ntk], pss[:, :ntk], AF.Exp,
                                         scale=c.SCALE, accum_out=den[:])
                    nc.vector.reciprocal(den[:], den[:])
                    nc.vector.tensor_scalar_mul(A[:, :ntk], A[:, :ntk],
                                                den[:])
                    for tk in range(tq + 1 if causal else c.NT):
                        pt = ps_tr.tile([P, P], BF, tag="ps_tr", name="pt")
                        nc.tensor.transpose(pt[:], A[:, tk * P:(tk + 1) * P],
                                            ident_b[:])
                        nc.vector.tensor_copy(
                            out=at_tiles[tk][:, tq * P:(tq + 1) * P],
                            in_=pt[:])
                for tk in range(c.NT):
                    cols0 = tk * P if causal else 0
                    nc.tensor.matmul(
                        ps_pair[po:po + 64, cols0:c.T],
                        v_nat[:, tk * c.DL + h * c.HD:
                              tk * c.DL + h * c.HD + 64],
                        at_tiles[tk][:, cols0:c.T],
                        start=(tk == 0), stop=(tk == c.NT - 1))
            nc.scalar.copy(out=attn_T[:, hp * c.T:(hp + 1) * c.T],
                           in_=ps_pair[:])

    NB = max(c.D // 512, 1)
    NW = min(512, c.D)

    def mm_to_natural(src_T, nk, wts, bname):
        """[T, D] = src_T.T @ W, returned as per-t bf16 [P, D] tiles."""
        parts = []
        for t in range(c.NT):
            sb = scratch.tile([P, c.D], BF, tag="oproj", name="sb", bufs=3)
            for n in range(NB):
                ps = ps_proj.tile([P, NW], F32, tag="ps_proj", name="ps")
                for k in range(nk):
                    nc.tensor.matmul(
                        ps[:], src_T[:, k * c.T + t * P: k * c.T + t * P + P],
                        wts[k][:, n * NW:(n + 1) * NW],
                        start=(k == 0), stop=(k == nk - 1))
                nc.scalar.copy(out=sb[:, n * NW:(n + 1) * NW], in_=ps[:])
            if bname in opt_sb:
                nc.vector.tensor_tensor(out=sb[:], in0=sb[:],
                                        in1=opt_sb[bname][:], op=ALU.add)
            parts.append(sb)
        return parts

    def out_proj(wname, bname):
        wts = []
        for k in range(c.NO):
            wt = wpool.tile([P, c.D], BF, tag=f"wo_{k}", name="wt", bufs=1)
            nc.sync.dma_start(out=wt[:], in_=w[wname][k * P:(k + 1) * P, :])
            wts.append(wt)
        return mm_to_natural(attn_T, c.NO, wts, bname)

    def ffn(w1name, b1name, w2name, b2name):
        for m in range(c.NFF):
            wt = wpool.tile([P, c.ND * P], BF, tag="wff1", name="wt")
            nc.sync.dma_start(
                out=wt[:].rearrange("p (nd m) -> p nd m", m=P),
                in_=w[w1name][:, m * P:(m + 1) * P]
                .rearrange("(nd p) m -> p nd m", p=P))
            ps = ps_proj.tile([P, c.T], F32, tag="ps_proj", name="ps")
            for k in range(c.ND):
                nc.tensor.matmul(ps[:], wt[:, k * P:(k + 1) * P],
                                 x_T[:, k * c.T:(k + 1) * c.T],
                                 start=(k == 0), stop=(k == c.ND - 1))
            if b1name in opt_sb:
                nc.scalar.activation(h_T[:, m * c.T:(m + 1) * c.T], ps[:],
                                     AF.Relu,
                                     bias=opt_sb[b1name][:, m:m + 1])
            else:
                nc.scalar.activation(h_T[:, m * c.T:(m + 1) * c.T], ps[:],
                                     AF.Relu)
        wts = []
        for k in range(c.NFF):
            wt = wpool.tile([P, c.D], BF, tag=f"wff2_{k}", name="wt", bufs=1)
            nc.sync.dma_start(out=wt[:], in_=w[w2name][k * P:(k + 1) * P, :])
            wts.append(wt)
        return mm_to_natural(h_T, c.NFF, wts, b2name)

    # ---------------- encoder ----------------
    embed(codes_in)
    for l in range(c.L):
        transpose_x_into(x_T, nc.vector)
        proj_rows(f"e_qkv_{l}", f"e_qkv_b_{l}", x_T, 0, q_T)
        proj_rows(f"e_qkv_{l}", f"e_qkv_b_{l}", x_T, c.DL, k_T)
        proj_v(f"e_qkv_{l}", None, x_T)
        attention(causal=False)
        allreduce_residual(out_proj(f"e_out_{l}", f"e_out_b_{l}"))
        layernorm(f"e_ln1_w_{l}", f"e_ln1_b_{l}")
        transpose_x_into(x_T, nc.vector)
        allreduce_residual(
            ffn(f"e_ff1_{l}", f"e_ff1_b_{l}", f"e_ff2_{l}", f"e_ff2_b_{l}"))
        layernorm(f"e_ln2_w_{l}", f"e_ln2_b_{l}")
    transpose_x_into(mem_T, nc.scalar)

    # ---------------- decoder ----------------
    embed(codes_tgt)
    for l in range(c.L):
        transpose_x_into(x_T, nc.vector)
        proj_rows(f"d_sqkv_{l}", f"d_sqkv_b_{l}", x_T, 0, q_T)
        proj_rows(f"d_sqkv_{l}", f"d_sqkv_b_{l}", x_T, c.DL, k_T)
        proj_v(f"d_sqkv_{l}", None, x_T)
        attention(causal=True)
        allreduce_residual(out_proj(f"d_sout_{l}", f"d_sout_b_{l}"))
        layernorm(f"d_ln1_w_{l}", f"d_ln1_b_{l}")
        transpose_x_into(x_T, nc.vector)
        proj_rows(f"d_cqkv_{l}", f"d_cqkv_b_{l}", x_T, 0, q_T)
        proj_rows(f"d_cqkv_{l}", f"d_cqkv_b_{l}", mem_T, c.DL, k_T)
        proj_v(f"d_cqkv_{l}", None, mem_T)
        attention(causal=False)
        allreduce_residual(out_proj(f"d_cout_{l}", f"d_cout_b_{l}"))
        layernorm(f"d_ln2_w_{l}", f"d_ln2_b_{l}")
        transpose_x_into(x_T, nc.vector)
        allreduce_residual(
            ffn(f"d_ff1_{l}", f"d_ff1_b_{l}", f"d_ff2_{l}", f"d_ff2_b_{l}"))
        layernorm(f"d_ln3_w_{l}", f"d_ln3_b_{l}")

    # ---------------- output head (all Q on every core) ----------------
    transpose_x_into(x_T, nc.vector)   # x_T now holds y_T
    NBV = max(c.V // 512, 1)
    NWV = min(512, c.V)
    for j in range(c.Q):
        hw = wpool.tile([P, c.V], BF, tag="whead", name="hw", bufs=2)
        nc.sync.dma_start(out=hw[:],
                          in_=w["head_t"][:, j * c.V:(j + 1) * c.V])
        for t in range(c.NT):
            sb = scratch.tile([P, c.V], F32, tag="lgt", name="sb", bufs=3)
            for n in range(NBV):
                ps = ps_proj.tile([P, NWV], F32, tag="ps_proj", name="ps")
                nc.tensor.matmul(
                    ps[:], x_T[:, j * c.T + t * P: j * c.T + t * P + P],
                    hw[:, n * NWV:(n + 1) * NWV], start=True, stop=True)
                nc.scalar.copy(out=sb[:, n * NWV:(n + 1) * NWV], in_=ps[:])
            if "head_b" in opt_sb:
                nc.vector.tensor_tensor(
                    out=sb[:], in0=sb[:],
                    in1=opt_sb["head_b"][:, j * c.V:(j + 1) * c.V],
                    op=ALU.add)
            nc.sync.dma_start(out=logits[j, t * P:(t + 1) * P, :], in_=sb[:])

    es.close()


# --------------------------------------------------------------------------
# host side
# --------------------------------------------------------------------------

_PROG_CACHE = {}


def parse_cfg(inputs, TP=None, n_cores=None):
    B, Q, T = inputs["input_codes"].shape
    _, V, E = np.asarray(inputs["tok_emb"]).shape
    L, _, D = np.asarray(inputs["e_qkv_w"]).shape
    FF = np.asarray(inputs["e_ff1_w"]).shape[1]
    H = D // 64
    if TP is None:
        TP = int(os.environ.get("BASS_S2S_TP", "2"))
    if n_cores is None:
        n_cores = B * TP
    flags = set()
    for l in range(L):
        for ref, knm in [("e_qkv_b", "e_qkv_b"), ("d_sqkv_b", "d_sqkv_b"),
                         ("d_cqkv_b", "d_cqkv_b"), ("e_ff1_b", "e_ff1_b"),
                         ("d_ff1_b", "d_ff1_b"), ("e_out_b", "e_out_b"),
                         ("e_ff2_b", "e_ff2_b"), ("d_sout_b", "d_sout_b"),
                         ("d_cout_b", "d_cout_b"), ("d_ff2_b", "d_ff2_b")]:
            if np.any(np.asarray(inputs[ref])[l]):
                flags.add(f"{knm}_{l}")
        for ln in ["e_ln1", "e_ln2", "d_ln1", "d_ln2", "d_ln3"]:
            if not np.all(np.asarray(inputs[ln + "_w"])[l] == 1.0):
                flags.add(f"{ln}_w_{l}")
            if np.any(np.asarray(inputs[ln + "_b"])[l]):
                flags.add(f"{ln}_b_{l}")
    if np.any(np.asarray(inputs["head_b"])):
        flags.add("head_b")
    # v-bias unsupported in-kernel; fall back assertion
    for l in range(L):
        for nm in ["e_qkv_b", "d_sqkv_b", "d_cqkv_b"]:
            vb = np.asarray(inputs[nm])[l][2 * D:3 * D]
            assert not np.any(vb), "nonzero v bias not supported"
    return Cfg(B, Q, T, D, H, V, L, FF, TP, n_cores, flags)


def build_inmaps(inputs, c: Cfg):
    g = lambda nm: np.asarray(inputs[nm], np.float32)
    bf = lambda a: np.ascontiguousarray(a, dtype=np.float32).astype(BF16)

    tok = np.asarray(inputs["tok_emb"], np.float32)
    posf = np.ascontiguousarray(g("pos_emb")[0, :c.T, :])
    head_w = g("head_w")
    head_t = np.concatenate([head_w[q].T for q in range(c.Q)], axis=1)

    common = {f"tok_emb_{q}": np.ascontiguousarray(tok[q])
              for q in range(c.Q)}
    common["pos"] = posf
    common["head_t"] = bf(head_t)
    if "head_b" in c.flags:
        hb = g("head_b").reshape(-1)
        common["head_b"] = np.broadcast_to(hb, (P, c.Q * c.V)).copy()

    per_tp = []
    for tp in range(c.TP):
        d = {}
        sl_d = slice(tp * c.DL, (tp + 1) * c.DL)
        sl_f = slice(tp * c.FFL, (tp + 1) * c.FFL)
        for pre, wq, wo, w1, w2 in [
                ("e", "e_qkv_w", "e_out_w", "e_ff1_w", "e_ff2_w"),
                ("d_s", "d_sqkv_w", "d_sout_w", None, None),
                ("d_c", "d_cqkv_w", "d_cout_w", None, None)]:
            qkv = g(wq)
            out_w = g(wo)
            for l in range(c.L):
                wqkv = np.concatenate(
                    [qkv[l][0:c.D][sl_d], qkv[l][c.D:2 * c.D][sl_d],
                     qkv[l][2 * c.D:3 * c.D][sl_d]], axis=0)
                nm = {"e": "e_qkv", "d_s": "d_sqkv", "d_c": "d_cqkv"}[pre]
                d[f"{nm}_{l}"] = bf(wqkv.T)
                onm = {"e": "e_out", "d_s": "d_sout", "d_c": "d_cout"}[pre]
                d[f"{onm}_{l}"] = bf(out_w[l][:, sl_d].T)
        for l in range(c.L):
            d[f"e_ff1_{l}"] = bf(g("e_ff1_w")[l][sl_f].T)
            d[f"e_ff2_{l}"] = bf(g("e_ff2_w")[l][:, sl_f].T)
            d[f"d_ff1_{l}"] = bf(g("d_ff1_w")[l][sl_f].T)
            d[f"d_ff2_{l}"] = bf(g("d_ff2_w")[l][:, sl_f].T)
        # optional biases
        for l in range(c.L):
            for knm, ref in [("e_qkv_b", "e_qkv_b"), ("d_sqkv_b", "d_sqkv_b"),
                             ("d_cqkv_b", "d_cqkv_b")]:
                if f"{knm}_{l}" in c.flags:
                    b = g(ref)[l]
                    d[f"{knm}_{l}"] = np.concatenate(
                        [b[0:c.D][sl_d], b[c.D:2 * c.D][sl_d],
                         np.zeros(c.DL, np.float32)])
            for knm in ["e_ff1_b", "d_ff1_b"]:
                if f"{knm}_{l}" in c.flags:
                    d[f"{knm}_{l}"] = np.ascontiguousarray(g(knm + "")[l][sl_f])
            for knm in ["e_out_b", "e_ff2_b", "d_sout_b", "d_cout_b",
                        "d_ff2_b"]:
                if f"{knm}_{l}" in c.flags:
                    d[f"{knm}_{l}"] = np.broadcast_to(
                        g(knm)[l], (P, c.D)).copy()
            for ln in ["e_ln1", "e_ln2", "d_ln1", "d_ln2", "d_ln3"]:
                for sfx in ["w", "b"]:
                    if f"{ln}_{sfx}_{l}" in c.flags:
                        d[f"{ln}_{sfx}_{l}"] = np.broadcast_to(
                            g(f"{ln}_{sfx}")[l], (P, c.D)).copy()
        per_tp.append(d)

    codes_in = np.asarray(inputs["input_codes"], np.int32)
    codes_tgt = np.asarray(inputs["target_codes"], np.int32)
    in_maps = []
    for core in range(c.n_cores):
        b, tp = core // c.TP, core % c.TP
        m = dict(common)
        m.update(per_tp[tp])
        m["codes_in"] = np.ascontiguousarray(codes_in[b % c.B])
        m["codes_tgt"] = np.ascontiguousarray(codes_tgt[b % c.B])
        in_maps.append(m)
    return in_maps


def postprocess(results, c: Cfg):
    out = np.empty((c.B, c.T, c.Q, c.V), np.float32)
    for b in range(c.B):
        r = results[b * c.TP]["logits"]      # [Q, T, V]
        out[b] = r.transpose(1, 0, 2)
    return out


def run(inputs, trace=False):
    from concourse.bass_utils import run_bass_kernel_spmd
    c = parse_cfg(inputs)
    key = c.key()
    if key not in _PROG_CACHE:
        _PROG_CACHE[key] = build_program(c)
    nc = _PROG_CACHE[key]
    in_maps = build_inmaps(inputs, c)
    res = run_bass_kernel_spmd(nc, in_maps, list(range(c.n_cores)),
                               trace=trace)
    return postprocess(res.results, c), res


def kernel(**inputs):
    out, _ = run(inputs, trace=False)
    return out
